# revision 41
# baseline (speedup 1.0000x reference)
"""AttentionRGCN layer on 8 Trainium2 NeuronCores (Bass/Tile) — v2.

Math notes (vs the jax reference):
  - alpha = exp(leaky_relu(score)) normalized over axis=1 of an [E,1] tensor
    is exactly 1.0 for every edge, so the attention branch (W3, a) drops out.
  - matmul commutes with segment_sum:
        segsum((h[src]+rel[etype]) @ Wn, dst) = segsum(h[src], dst) @ Wn
                                                + C @ (rel @ Wn)
    where C[n, r] counts (dst=n, etype=r) edges; C and rel@Wn are built
    host-side.  Per node: out = relu((S + C@relw)) ... concretely
        out = relu(S @ Wn + C @ relw) + (indeg>0 ? h@Lw : h@Ew),
    S = segsum(h[src], dst).

Design (v2):
  - dst-node sharding over 8 cores; per-core slots = ceil(n/(8*128))*128.
  - Edges per core are split into two gather arenas by src half (int16
    gather index limit), each sorted by (dst_slot, src), padded to a
    cross-core-uniform multiple of 128 with dummy edges into a dummy slot.
  - h[src] rows are fetched bf16 (512B descriptors) with a few BIG
    gpsimd.dma_gather calls per arena (desc-gen on the Q7 cluster is
    serialized per call with ~2-6us fixed overhead, so few big calls keep
    the DMA engines fed instead of starving them 98 times).
  - Nodes are packed into slots so the per-slot prefix degree tracks a
    linear target (both arenas jointly); every 128-edge gather group then
    scatters into a narrow, statically-known dst window (~16-40 slots).
    The scatter is a TensorE matmul: stationary = gathered [128, 128feat]
    slice, moving = narrow fp8 one-hot.  The first matmul of each 512-slot
    PSUM window is widened to the full window with start=True (zero-fill).
  - Schedule (group windows, call boundaries) is computed from the union
    over cores, so one SPMD graph serves all 8 cores; only tensor data
    differs per core.
  - Epilogue per 512-slot window: agg = Wn^T@S^T + relw^T@C^T (+ loop
    message Lw^T@h^T), relu, add, DMA out.  Everything stays in the
    feature-on-partition orientation; no on-device transposes.
"""

import sys

for _p in ("/opt/trn_rl_repo",):
    if _p not in sys.path:
        sys.path.append(_p)

import numpy as np
import ml_dtypes

from concourse import bacc, bass, mybir, tile
from concourse import library_config
from concourse.bass_utils import run_bass_kernel_spmd

P = 128
D = 256  # feature dim (fixed by problem)
NCORES = 8
BF16 = mybir.dt.bfloat16
F32 = mybir.dt.float32
I16 = mybir.dt.int16
FP8 = mybir.dt.float8e4
np_bf16 = ml_dtypes.bfloat16
np_fp8 = ml_dtypes.float8_e4m3


# ----------------------------------------------------------------------------
# Host-side sharding / packing (index work only -- no float math on h)
# ----------------------------------------------------------------------------

def _order_nodes_quantile(nodes, deg_lo, deg_hi, rl=None, rh=None):
    """Order `nodes` so the prefix sums of (deg_lo, deg_hi) track linear
    targets.  Keeps every 128-edge dst-sorted group inside a narrow slot
    window.  Greedy: at each slot pick the remaining (dl, dh) bucket that
    minimizes the max prefix deviation.  Pass COMMON rates (rl, rh) so all
    cores' prefixes track the same line (narrow cross-core window union)."""
    n = len(nodes)
    if n == 0:
        return nodes
    dl = deg_lo[nodes].astype(np.float64)
    dh = deg_hi[nodes].astype(np.float64)
    el, eh = dl.sum(), dh.sum()
    if rl is None:
        rl = el / n
    if rh is None:
        rh = eh / n
    # bucket nodes by (dl, dh)
    keys, inv = np.unique(np.stack([dl, dh], 1), axis=0, return_inverse=True)
    order_in_bucket = np.argsort(inv, kind="stable")
    bucket_start = np.searchsorted(inv[order_in_bucket], np.arange(len(keys)))
    bucket_cnt = np.bincount(inv, minlength=len(keys)).astype(np.int64)
    bucket_used = np.zeros(len(keys), np.int64)
    kl = keys[:, 0]
    kh = keys[:, 1]
    out = np.empty(n, np.int64)
    cl = ch = 0.0
    for s in range(n):
        tl = rl * (s + 1)
        th = rh * (s + 1)
        devl = np.abs(cl + kl - tl)
        devh = np.abs(ch + kh - th)
        score = np.maximum(devl, devh)
        score[bucket_used >= bucket_cnt] = np.inf
        b = int(np.argmin(score))
        idx = order_in_bucket[bucket_start[b] + bucket_used[b]]
        bucket_used[b] += 1
        out[s] = nodes[idx]
        cl += kl[b]
        ch += kh[b]
    return out


def host_prep(h, rel_emb, weight_neighbor, loop_weight, evolve_loop_weight,
              src, dst, etype, ncores=NCORES):
    """Build per-core device input arrays + the static schedule."""
    n_nodes, d = h.shape
    assert d == D
    n_rel = rel_emb.shape[0]
    e = src.shape[0]
    src = np.asarray(src).astype(np.int64)
    dst = np.asarray(dst).astype(np.int64)
    etype = np.asarray(etype).astype(np.int64)

    split = min((n_nodes + 1) // 2, 32000)
    # per-core slot count, with >=8 dummy slots per core guaranteed
    slots = -(-(n_nodes + 8 * ncores) // (ncores * P)) * P
    assert n_nodes <= ncores * (slots - 8), "need >=8 dummy slots per core"

    e_half = (src >= split).astype(np.int64)
    deg_lo = np.bincount(dst[e_half == 0], minlength=n_nodes)
    deg_hi = np.bincount(dst[e_half == 1], minlength=n_nodes)
    deg = deg_lo + deg_hi
    z_needed = bool((deg == 0).any())

    # ---- node -> core: greedy balance total degree, cap node count ------
    cap_nodes = slots - 8
    order = np.argsort(-deg, kind="stable")
    core_load = np.zeros(ncores, np.int64)
    core_cnt = np.zeros(ncores, np.int64)
    core_of = np.empty(n_nodes, np.int64)
    for nid in order:
        ld = core_load.astype(np.float64).copy()
        ld[core_cnt >= cap_nodes] = np.inf
        c = int(np.argmin(ld))
        core_of[nid] = c
        core_load[c] += deg[nid]
        core_cnt[c] += 1

    # ---- node -> slot within core: prefix-quantile ordering -------------
    # Common target rates across cores so group->slot mappings align.
    el_c = np.zeros(ncores, np.float64)
    eh_c = np.zeros(ncores, np.float64)
    for c in range(ncores):
        m = core_of == c
        el_c[c] = deg_lo[m].sum()
        eh_c[c] = deg_hi[m].sum()
    n_avg = n_nodes / ncores
    rl_common = el_c.max() / n_avg
    rh_common = eh_c.max() / n_avg
    node_of_slot = np.full((ncores, slots), -1, np.int64)
    n_real = np.zeros(ncores, np.int64)
    for c in range(ncores):
        nodes_c = np.where(core_of == c)[0]
        seq = _order_nodes_quantile(nodes_c, deg_lo, deg_hi,
                                    rl_common, rh_common)
        node_of_slot[c, : len(seq)] = seq
        n_real[c] = len(seq)

    slot_of_node = np.full(n_nodes, -1, np.int64)
    flat = node_of_slot.reshape(-1)
    valid = flat >= 0
    slot_of_node[flat[valid]] = np.arange(ncores * slots)[valid]
    assert np.all(slot_of_node >= 0)

    gslot = slot_of_node[dst]
    e_core = gslot // slots
    e_local = gslot % slots

    # ---- per-(core, arena) edge lists sorted by (slot, src) -------------
    # padded to uniform count (x128) with dummy edges -> first dummy slot
    e_cnt = np.zeros((ncores, 2), np.int64)
    np.add.at(e_cnt, (e_core, e_half), 1)
    e_pad = [int(-(-e_cnt[:, a].max() // P) * P) for a in range(2)]

    # arrays per (core, arena): src index (rebased), slot
    a_src = [np.zeros((ncores, e_pad[a]), np.int64) for a in range(2)]
    a_slot = [np.zeros((ncores, e_pad[a]), np.int64) for a in range(2)]
    for c in range(ncores):
        for a in range(2):
            m = (e_core == c) & (e_half == a)
            es = e_local[m]
            ss = src[m] - (split if a else 0)
            o = np.lexsort((ss, es))
            cnt = len(es)
            a_src[a][c, :cnt] = ss[o]
            a_slot[a][c, :cnt] = es[o]
            # pad edges: src row 0 of the arena, dst = first dummy slot
            a_src[a][c, cnt:] = 0
            a_slot[a][c, cnt:] = n_real[c]

    n_grp = [e_pad[a] // P for a in range(2)]

    # ---- group windows (union over cores) -------------------------------
    win_lo = [None, None]
    win_w = [None, None]
    for a in range(2):
        sl = a_slot[a].reshape(ncores, n_grp[a], P)
        lo = sl.min(axis=2).min(axis=0)
        hi = sl.max(axis=2).max(axis=0)
        win_lo[a] = lo
        win_w[a] = hi - lo + 1

    # ---- output windows (512-slot chunks, last ragged) ------------------
    windows = []
    w0 = 0
    while w0 < slots:
        wl = min(512, slots - w0)
        windows.append((w0, wl))
        w0 += wl
    nwin = len(windows)

    # ---- sub-matmul schedule per window ---------------------------------
    # sub = (arena, group, col0_abs, width, oh_off, widened)
    subs_by_win = [[] for _ in range(nwin)]
    for a in range(2):
        for g in range(n_grp[a]):
            lo = int(win_lo[a][g])
            hi = lo + int(win_w[a][g]) - 1
            wi0 = lo // 512
            wi1 = min(hi // 512, nwin - 1)
            for wi in range(wi0, wi1 + 1):
                wb, wl = windows[wi]
                c0 = max(lo, wb)
                c1 = min(hi, wb + wl - 1)
                subs_by_win[wi].append((a, g, c0, c1 - c0 + 1))
    for wi in range(nwin):
        assert subs_by_win[wi], f"window {wi} has no groups"
        subs_by_win[wi].sort(key=lambda s: (s[2], s[0], s[1]))

    # assign one-hot arena offsets; widen the first sub of each window.
    # Column ranges are aligned to 8 slots and oh offsets to 16 bytes
    # (defensive: PE moving-operand / PSUM access alignment).
    import os as _os
    widen_all = bool(_os.environ.get("KFULLW"))
    oh_off = 0
    schedule = []  # per window: list of (a, g, col0, width, off, widened)
    for wi in range(nwin):
        wb, wl = windows[wi]
        lst = []
        for si, (a, g, c0, wdt) in enumerate(subs_by_win[wi]):
            oh_off = -(-oh_off // 16) * 16
            if si == 0 or widen_all:
                lst.append((a, g, wb, wl, oh_off, si == 0))
                oh_off += wl
            else:
                c0a = wb + ((c0 - wb) & ~7)
                enda = min(wb + wl, c0a + -(-(c0 + wdt - c0a) // 8) * 8)
                lst.append((a, g, c0a, enda - c0a, oh_off, False))
                oh_off += enda - c0a
        schedule.append(lst)
    oh_cols = -(-oh_off // 16) * 16

    # ---- one-hot arena data ---------------------------------------------
    oh = np.zeros((ncores, P, oh_cols), np.float32)
    for wi in range(nwin):
        for (a, g, c0, wdt, off, widened) in schedule[wi]:
            sl = a_slot[a][:, g * P:(g + 1) * P]  # [ncores, P]
            rel_col = sl - c0
            ok = (rel_col >= 0) & (rel_col < wdt)
            ci, li = np.nonzero(ok)
            oh[ci, li, off + rel_col[ci, li]] += 1.0
    assert oh.max() <= 16, "fp8e4m3 exact-integer range exceeded"

    # ---- gather call segmentation: one call per (arena, window) ---------
    # call wi covers the groups first needed by window wi; issued right
    # after window wi-1's scatter so bufs=3 recycling is read-safe.
    need = np.zeros((2, nwin), np.int64)
    for wi in range(nwin):
        for (a, g, _c0, _w, _off, _wd) in schedule[wi]:
            need[a, wi] = max(need[a, wi], g)
    for a in range(2):
        for wi in range(1, nwin):
            need[a, wi] = max(need[a, wi], need[a, wi - 1])
        need[a, nwin - 1] = n_grp[a] - 1

    # Calls chop the group range into GMAX-group pieces regardless of
    # window boundaries (the Q7 gather ucode rejects calls beyond ~1000
    # idxs; 896 = 7*128 is the proven-safe size).  call_win = the window
    # at which the call's FIRST group is first needed, used to schedule
    # the issue two windows ahead.
    GMAX = 7
    first_need = [np.zeros(n_grp[a], np.int64) for a in range(2)]
    for a in range(2):
        for wi in range(nwin - 1, -1, -1):
            prev = int(need[a, wi - 1]) + 1 if wi > 0 else 0
            first_need[a][prev:int(need[a, wi]) + 1] = wi
    calls = [[], []]      # per arena: flat list of (g0, ng) sub-calls
    call_win = [[], []]   # window index of each sub-call (issue schedule)
    for a in range(2):
        g0 = 0
        while g0 < n_grp[a]:
            ng = min(GMAX, n_grp[a] - g0)
            if g0 + ng >= n_grp[a] and ng > 2:
                # split the FINAL call so the truly-last desc-gen gates
                # only ~2 groups of scatter work (shrinks the tail)
                calls[a].append((g0, ng - 2))
                call_win[a].append(int(first_need[a][g0]))
                g0 += ng - 2
                ng = 2
            calls[a].append((g0, ng))
            call_win[a].append(int(first_need[a][g0]))
            g0 += ng
    cg_max = GMAX

    # chunk map: group -> (call idx, idx within call)
    chunk_of = [np.zeros((n_grp[a], 2), np.int64) for a in range(2)]
    for a in range(2):
        for ci, (g0, ng) in enumerate(calls[a]):
            for j in range(ng):
                chunk_of[a][g0 + j] = (ci, j)

    # ---- wrapped gather indexes (per call segment) ----------------------
    gidx = []
    for a in range(2):
        cols = []
        for (g0, ng) in calls[a]:
            if ng == 0:
                continue
            seg = a_src[a][:, g0 * P:(g0 + ng) * P]  # [ncores, ng*P]
            w = seg.reshape(ncores, -1, 16).transpose(0, 2, 1)  # [nc,16,n/16]
            cols.append(np.tile(w, (1, 8, 1)))
        gidx.append(np.concatenate(cols, axis=2).astype(np.int16))

    # ---- C^T counts (real edges only) -----------------------------------
    ct = np.zeros((ncores, n_rel, slots), np.float32)
    np.add.at(ct, (e_core, etype, e_local), 1.0)
    assert ct.max() <= 16, "fp8e4m3 exact-integer range exceeded"

    # ---- own h rows, pre-transposed [D, slots] --------------------------
    h_ownT = np.zeros((ncores, D, slots), np.float32)
    for c in range(ncores):
        ns = node_of_slot[c]
        v = ns >= 0
        h_ownT[c][:, v] = h[ns[v]].T

    # ---- weights (host-cast bf16) + relw = rel @ Wn ---------------------
    relw = (rel_emb.astype(np.float64) @ weight_neighbor.astype(np.float64))
    relw = relw.astype(np.float32)

    selm = None
    if z_needed:
        selm = np.ones((ncores, P, slots), np.float32)
        for c in range(ncores):
            ns = node_of_slot[c]
            good = (ns >= 0) & (deg[np.maximum(ns, 0)] > 0)
            selm[c] = np.tile(good[None, :], (P, 1))

    h_lo_bf = np.ascontiguousarray(h[:split]).astype(np_bf16)
    h_hi_bf = np.ascontiguousarray(h[split:]).astype(np_bf16)
    in_maps = []
    for c in range(ncores):
        m = {
            "h_lo": h_lo_bf,
            "h_hi": h_hi_bf,
            "h_ownt": h_ownT[c].astype(np_bf16),
            "gidx_lo": gidx[0][c],
            "gidx_hi": gidx[1][c],
            "oh": oh[c].astype(np_fp8),
            "ct": ct[c].astype(np_fp8),
            "relw": relw.astype(np_bf16),
            "w_n": weight_neighbor.astype(np_bf16),
            "w_l": loop_weight.astype(np_bf16),
        }
        if z_needed:
            m["w_e"] = evolve_loop_weight.astype(np_bf16)
            m["selm"] = selm[c].astype(np_bf16)
        in_maps.append(m)

    # hashable schedule signature for the graph cache
    sig = (n_nodes, n_rel, split, slots, z_needed, tuple(e_pad), oh_cols,
           tuple(tuple(cl) for cl in (tuple(c) for c in calls[0])),
           tuple(tuple(cl) for cl in (tuple(c) for c in calls[1])),
           tuple(call_win[0]), tuple(call_win[1]),
           tuple(tuple(s) for w in schedule for s in w))

    meta = dict(
        n_nodes=n_nodes, n_rel=n_rel, split=split, slots=slots,
        z_needed=z_needed, e_pad=e_pad, n_grp=n_grp, oh_cols=oh_cols,
        windows=windows, schedule=schedule, calls=calls, chunk_of=chunk_of,
        call_win=call_win, node_of_slot=node_of_slot, cg_max=cg_max, sig=sig,
    )
    return in_maps, meta


# ----------------------------------------------------------------------------
# Device graph
# ----------------------------------------------------------------------------

def build_graph(meta, ncores=NCORES):
    n_nodes = meta["n_nodes"]
    n_rel = meta["n_rel"]
    split = meta["split"]
    slots = meta["slots"]
    z_needed = meta["z_needed"]
    e_pad = meta["e_pad"]
    n_grp = meta["n_grp"]
    oh_cols = meta["oh_cols"]
    windows = meta["windows"]
    schedule = meta["schedule"]
    calls = meta["calls"]
    chunk_of = meta["chunk_of"]
    call_win = meta["call_win"]
    cg_max = meta["cg_max"]
    nwin = len(windows)
    # call indices to issue per (arena, window)
    calls_of_win = [[[] for _ in range(nwin)] for _ in range(2)]
    for a in range(2):
        for ci, wi in enumerate(call_win[a]):
            calls_of_win[a][wi].append(ci)

    nc = bacc.Bacc("TRN2", target_bir_lowering=False, debug=False,
                   num_devices=ncores, num_swdge_queues=4)

    dt = nc.dram_tensor
    h_lo = dt("h_lo", [split, D], BF16, kind="ExternalInput")
    h_hi = dt("h_hi", [n_nodes - split, D], BF16, kind="ExternalInput")
    h_ownt = dt("h_ownt", [D, slots], BF16, kind="ExternalInput")
    gidx = [dt("gidx_lo", [P, e_pad[0] // 16 * 8 // 8], I16,
               kind="ExternalInput"),
            dt("gidx_hi", [P, e_pad[1] // 16 * 8 // 8], I16,
               kind="ExternalInput")]
    ohd = dt("oh", [P, oh_cols], FP8, kind="ExternalInput")
    ctd = dt("ct", [n_rel, slots], FP8, kind="ExternalInput")
    relwd = dt("relw", [n_rel, D], BF16, kind="ExternalInput")
    w_nd = dt("w_n", [D, D], BF16, kind="ExternalInput")
    w_ld = dt("w_l", [D, D], BF16, kind="ExternalInput")
    if z_needed:
        w_ed = dt("w_e", [D, D], BF16, kind="ExternalInput")
        selmd = dt("selm", [P, slots], BF16, kind="ExternalInput")
    out = dt("out", [2, P, slots], BF16, kind="ExternalOutput")

    rel_k = [(k, min(P, n_rel - k)) for k in range(0, n_rel, P)]
    h_src_dram = [h_lo, h_hi]

    with tile.TileContext(nc) as tc:
        # gather buffer depth: must cover all in-flight sub-calls across 3
        # consecutive windows (issue lookahead 2 + current consumption)
        max_subs3 = 4
        for a in range(2):
            for wi in range(nwin):
                s = sum(len(calls_of_win[a][w])
                        for w in range(wi, min(wi + 3, nwin)))
                max_subs3 = max(max_subs3, s)
        gbufs = max_subs3 + 1

        with (
            tc.tile_pool(name="persist", bufs=1) as pp,
            tc.tile_pool(name="ga0", bufs=gbufs) as gp0,
            tc.tile_pool(name="ga1", bufs=gbufs) as gp1,
            tc.tile_pool(name="stp", bufs=3) as stp,
            tc.tile_pool(name="htp", bufs=3) as htp,
            tc.tile_pool(name="epi", bufs=4) as ep,
            tc.tile_pool(name="psum", bufs=8, space="PSUM") as psp,
        ):
            nc.gpsimd.load_library(library_config.mlp)
            gpools = [gp0, gp1]

            # gather indexes: one fat DMA per arena (per-partition rows of
            # ~5KB move at full rate; small sliced loads were 128x832B
            # packets taking ~12us)
            gidx_t = [pp.tile([P, e_pad[a] // 16], I16, tag=f"gidx{a}",
                              name=f"gidx_t{a}") for a in range(2)]
            nc.sync.dma_start(gidx_t[0][:], gidx[0].ap())
            nc.scalar.dma_start(gidx_t[1][:], gidx[1].ap())

            # warm-up gather: the first dma_gather invocation pays ~5us of
            # cold Q7 ucode cost; run a dummy 128-idx call (idxs memset to
            # row 0 -- no DMA dependency) so it overlaps the gidx load and
            # the real stream starts warm
            wu_idx = pp.tile([P, 8], I16, tag="wuidx")
            nc.vector.memset(wu_idx[:], 0)
            wu_out = pp.tile([P, 1, D], BF16, tag="wuout")
            nc.gpsimd.dma_gather(wu_out[:], h_lo.ap(), wu_idx[:],
                                 P, P, D, queue_num=0)

            # gather machinery ------------------------------------------------
            g_tiles = [{}, {}]          # arena -> call idx -> tile
            qctr = [0]
            # one shared register per distinct num_idxs (avoids a per-call
            # MOVE pacing the GpSimd queue at startup)
            nidx_regs = {}

            def issue_call(a, ci):
                g0, ng = calls[a][ci]
                if ng == 0:
                    return
                gf = gpools[a].tile([P, cg_max, D], BF16, tag=f"g{a}",
                                    name=f"gf_{a}_{ci}")
                n_idx = ng * P
                if n_idx not in nidx_regs:
                    nidx_regs[n_idx] = nc.gpsimd.to_reg(n_idx)
                off16 = g0 * P // 16
                nc.gpsimd.dma_gather(
                    gf[:, :ng, :], h_src_dram[a].ap(),
                    gidx_t[a][:, off16:off16 + n_idx // 16],
                    n_idx, nidx_regs[n_idx], D, queue_num=qctr[0] % 4,
                )
                qctr[0] += 1
                g_tiles[a][ci] = gf

            # prime: calls for windows 0,1 (no buffer recycling yet)
            for w in range(min(2, nwin)):
                for a in range(2):
                    for ci in calls_of_win[a][w]:
                        issue_call(a, ci)

            # static loads ----------------------------------------------------
            oh_t = pp.tile([P, oh_cols], FP8, tag="oh")
            half = (oh_cols // 2) // 2 * 2
            nc.sync.dma_start(oh_t[:, :half], ohd.ap()[:, :half])
            nc.scalar.dma_start(oh_t[:, half:], ohd.ap()[:, half:])

            ct_t = [pp.tile([kn, slots], FP8, tag=f"ct{k}", name=f"ct_t{k}")
                    for k, kn in rel_k]
            for i, (k, kn) in enumerate(rel_k):
                (nc.sync if i % 2 == 0 else nc.scalar).dma_start(
                    ct_t[i][:], ctd.ap()[k:k + kn, :])

            w_n_t = [pp.tile([P, D], BF16, tag=f"wn{kc}", name=f"wn{kc}")
                     for kc in range(2)]
            w_l_t = [pp.tile([P, D], BF16, tag=f"wl{kc}", name=f"wl{kc}")
                     for kc in range(2)]
            for kc in range(2):
                nc.sync.dma_start(w_n_t[kc][:], w_nd.ap()[kc * P:(kc + 1) * P, :])
                nc.scalar.dma_start(w_l_t[kc][:], w_ld.ap()[kc * P:(kc + 1) * P, :])
            relw_t = [pp.tile([kn, D], BF16, tag=f"relw{k}", name=f"relw{k}")
                      for k, kn in rel_k]
            for i, (k, kn) in enumerate(rel_k):
                nc.sync.dma_start(relw_t[i][:], relwd.ap()[k:k + kn, :])
            if z_needed:
                w_e_t = [pp.tile([P, D], BF16, tag=f"we{kc}", name=f"we{kc}")
                         for kc in range(2)]
                for kc in range(2):
                    nc.scalar.dma_start(w_e_t[kc][:],
                                        w_ed.ap()[kc * P:(kc + 1) * P, :])
                selm_t = pp.tile([P, slots], BF16, tag="selm")
                nc.sync.dma_start(selm_t[:], selmd.ap())

            # ---- main loop over windows -------------------------------------
            st_tiles = {}
            ht_tiles = {}
            lsb_pool = ep

            def emit_epilogue(wi):
                wb, wl = windows[wi]
                st_w = st_tiles.pop(wi)
                ht_w = ht_tiles.pop(wi)
                for fh in range(2):
                    agg = psp.tile([P, wl], F32, tag="ps",
                                   name=f"agg{wi}_{fh}")
                    nmm = 2 + len(rel_k)
                    i = 0
                    for kc in range(2):
                        nc.tensor.matmul(
                            agg[:], w_n_t[kc][:, fh * P:(fh + 1) * P],
                            st_w[:, kc, :wl],
                            start=(i == 0), stop=(i == nmm - 1))
                        i += 1
                    for ki, (k, kn) in enumerate(rel_k):
                        nc.tensor.matmul(
                            agg[:], relw_t[ki][:, fh * P:(fh + 1) * P],
                            ct_t[ki][:, wb:wb + wl],
                            start=(i == 0), stop=(i == nmm - 1))
                        i += 1
                    ot = ep.tile([P, 512], BF16, tag="ot",
                                 name=f"ot{wi}_{fh}")
                    nc.scalar.activation(ot[:, :wl], agg[:],
                                         mybir.ActivationFunctionType.Relu)
                    lp = agg  # reuse bank; Tile orders after the relu read
                    for kc in range(2):
                        nc.tensor.matmul(
                            lp[:], w_l_t[kc][:, fh * P:(fh + 1) * P],
                            ht_w[:, kc, :wl],
                            start=(kc == 0), stop=(kc == 1))
                    if not z_needed:
                        nc.vector.tensor_add(ot[:, :wl], ot[:, :wl], lp[:])
                    else:
                        ep_ps = psp.tile([P, wl], F32, tag="ps",
                                         name=f"eps{wi}_{fh}")
                        for kc in range(2):
                            nc.tensor.matmul(
                                ep_ps[:], w_e_t[kc][:, fh * P:(fh + 1) * P],
                                ht_w[:, kc, :wl],
                                start=(kc == 0), stop=(kc == 1))
                        lsb = lsb_pool.tile([P, 512], F32, tag="lsb",
                                            name=f"lsb{wi}_{fh}")
                        nc.vector.tensor_tensor(
                            lsb[:, :wl], lp[:], selm_t[:, wb:wb + wl],
                            op=mybir.AluOpType.mult)
                        nc.vector.tensor_add(ot[:, :wl], ot[:, :wl],
                                             lsb[:, :wl])
                        me = lsb_pool.tile([P, 512], F32, tag="me",
                                           name=f"me{wi}_{fh}")
                        nc.vector.tensor_tensor(
                            me[:, :wl], ep_ps[:], selm_t[:, wb:wb + wl],
                            op=mybir.AluOpType.mult)
                        nc.vector.tensor_tensor(
                            me[:, :wl], ep_ps[:], me[:, :wl],
                            op=mybir.AluOpType.subtract)
                        nc.vector.tensor_add(ot[:, :wl], ot[:, :wl],
                                             me[:, :wl])
                    (nc.sync if fh == 0 else nc.scalar).dma_start(
                        out.ap()[fh, :, wb:wb + wl], ot[:, :wl])

            for wi in range(nwin):
                wb, wl = windows[wi]
                # ht stream for this window (consumed by epilogue(wi))
                ht_w = htp.tile([P, 2, 512], BF16, tag="ht",
                                name=f"ht{wi}")
                for fh in range(2):
                    (nc.scalar if fh == 0 else nc.sync).dma_start(
                        ht_w[:, fh, :wl],
                        h_ownt.ap()[fh * P:(fh + 1) * P, wb:wb + wl])
                ht_tiles[wi] = ht_w

                # epilogue of the previous window BEFORE this window's
                # scatter: PE chews on it while gather(wi) completes
                if wi > 0:
                    emit_epilogue(wi - 1)

                st_w = stp.tile([P, 2, 512], BF16, tag="st", name=f"st{wi}")
                subs = schedule[wi]
                for fh in range(2):
                    ps = psp.tile([P, wl], F32, tag="ps",
                                  name=f"ps_{wi}_{fh}")
                    nsub = len(subs)
                    for si, (a, g, c0, wdt, off, widened) in enumerate(subs):
                        ci, j = chunk_of[a][g]
                        gf = g_tiles[a][int(ci)]
                        nc.tensor.matmul(
                            ps[:, c0 - wb:c0 - wb + wdt],
                            gf[:, int(j), fh * P:(fh + 1) * P],
                            oh_t[:, off:off + wdt],
                            start=(si == 0), stop=(si == nsub - 1),
                            skip_group_check=True,
                        )
                    nc.scalar.activation(
                        st_w[:, fh, :wl], ps[:],
                        mybir.ActivationFunctionType.Identity)
                st_tiles[wi] = st_w

                # prefetch window wi+2's calls AFTER this window's matmuls:
                # gbufs covers 3 windows of sub-calls, so the recycled
                # buffer's readers (scatter <= wi) are all emitted
                if wi + 2 < nwin:
                    for a in range(2):
                        for ci in calls_of_win[a][wi + 2]:
                            issue_call(a, ci)
            emit_epilogue(nwin - 1)

    nc.compile()
    return nc


# ----------------------------------------------------------------------------
# Entry point
# ----------------------------------------------------------------------------

_CACHE = {}


def _run(inputs, ncores=NCORES, trace=False):
    h = np.asarray(inputs["h"], np.float32)
    rel_emb = np.asarray(inputs["rel_emb"], np.float32)
    w_n = np.asarray(inputs["weight_neighbor"], np.float32)
    w_l = np.asarray(inputs["loop_weight"], np.float32)
    w_e = np.asarray(inputs["evolve_loop_weight"], np.float32)
    src = np.asarray(inputs["src"])
    dst = np.asarray(inputs["dst"])
    etype = np.asarray(inputs["etype"])

    in_maps, meta = host_prep(h, rel_emb, w_n, w_l, w_e, src, dst, etype,
                              ncores=ncores)

    key = meta["sig"]
    if key not in _CACHE:
        _CACHE[key] = build_graph(meta, ncores=ncores)
    nc = _CACHE[key]

    res = run_bass_kernel_spmd(nc, in_maps, core_ids=list(range(ncores)),
                               trace=trace)

    n_nodes = meta["n_nodes"]
    slots = meta["slots"]
    node_of_slot = meta["node_of_slot"]
    out_full = np.zeros((n_nodes, D), np.float32)
    for c in range(ncores):
        oc = np.asarray(res.results[c]["out"])  # [2, P, slots]
        oc = oc.reshape(2 * P, slots)           # [D(feature), slots]
        ns = node_of_slot[c]
        v = ns >= 0
        out_full[ns[v]] = oc[:, v].T
    return out_full, res, meta


def kernel(**inputs) -> np.ndarray:
    out, _, _ = _run(inputs)
    return out.astype(np.float32)


if __name__ == "__main__":
    # tiny smoke test with a synthetic small graph
    rng = np.random.default_rng(0)
    N_, E_, R_ = 2048, 16384, 16
    inputs = dict(
        h=rng.standard_normal((N_, D), dtype=np.float32),
        rel_emb=(rng.standard_normal((R_, D)) * 0.1).astype(np.float32),
        weight_neighbor=rng.standard_normal((D, D), dtype=np.float32) * 0.05,
        loop_weight=rng.standard_normal((D, D), dtype=np.float32) * 0.05,
        evolve_loop_weight=rng.standard_normal((D, D), dtype=np.float32) * 0.05,
        src=rng.integers(0, N_, E_),
        dst=rng.integers(0, N_, E_),
        etype=rng.integers(0, R_, E_),
    )
    out, res, meta = _run(inputs)
    # numpy reference
    S = np.zeros((N_, D), np.float32)
    np.add.at(S, inputs["dst"], inputs["h"][inputs["src"]]
              + inputs["rel_emb"][inputs["etype"]])
    indeg = np.bincount(inputs["dst"], minlength=N_)
    loopm = np.where((indeg > 0)[:, None],
                     inputs["h"] @ inputs["loop_weight"],
                     inputs["h"] @ inputs["evolve_loop_weight"])
    ref = np.maximum(S @ inputs["weight_neighbor"], 0) + loopm
    err = np.abs(out - ref).max() / (np.abs(ref).max() + 1e-9)
    print("small-graph rel err:", err)
    print("meta:", {k: meta[k] for k in
                    ("slots", "e_pad", "n_grp", "oh_cols", "z_needed")})
